# revision 13
# baseline (speedup 1.0000x reference)
"""ArcFace head (loss_fn) on 8 TRN2 NeuronCores.

Strategy (classification-parallel, per sharding hint):
  - weight [100000, 512] is row-normalized + transposed on host, sharded over
    num_classes: each core gets w_t [512, 12800] (12500 real cols + 0-pad).
  - each core computes its S*cosine shard [512, 12800] = (S*input).T ^T @ w_t
    with fp16 matmuls (1 PE cyc/row vs 4 for fp32; ~3e-4 rel err), streams
    it to DRAM,
    and in the same pass accumulates per-row sum(exp(out - shift)) via the
    ScalarEngine's fused exp+accum — shift is a host-computed per-row
    numerical-stability offset (max only shifts, it cancels in log_softmax).
  - host combines the 8 per-shard Z partials, swaps the target element's
    exp(cos) for exp(phi) (ArcFace margin patch, 512 elements), computes the
    NLL loss, and scatters the 512 patched logits into the gathered output.
"""

import math
import sys

import numpy as np

for _p in ("/opt/trn_rl_repo", "/root/.axon_site/_ro/trn_rl_repo"):
    if _p not in sys.path:
        sys.path.insert(0, _p)

S = 64.0
MARGIN = 0.5
COS_M = math.cos(MARGIN)
SIN_M = math.sin(MARGIN)
TH = math.cos(math.pi - MARGIN)
MM = math.sin(math.pi - MARGIN) * MARGIN

B = 512          # batch
E = 512          # embed
C = 100000       # num classes
NCORES = 8
CSH = C // NCORES        # 12500 classes per core
CPAD = 12500             # no padding: 25 tiles of 500
NTILE = 500              # matmul free dim (classes per tile)
CT = CPAD // NTILE       # 25 class tiles per core
KT = E // 128            # 4 contraction chunks
BT = B // 128            # 4 batch tiles
CHUNK_NT = 5             # ntiles per DMA chunk (2500 classes)

DTYPE_MODE = "fp16"   # "f32r" | "bf16" | "fp16"

_CACHE: dict = {}


def _build():
    import concourse.bass as bass
    import concourse.bacc as bacc
    import concourse.mybir as mybir
    import concourse.tile as tile

    f32 = mybir.dt.float32

    nc = bacc.Bacc("TRN2", target_bir_lowering=False, debug=False,
                   num_devices=NCORES)

    fmm = {"f32r": mybir.dt.float32r, "bf16": mybir.dt.bfloat16,
           "fp16": mybir.dt.float16}[DTYPE_MODE]
    xt_d = nc.dram_tensor("xt", [E, B], fmm, kind="ExternalInput")
    wt_d = nc.dram_tensor("wt", [E, CPAD], fmm, kind="ExternalInput")
    ns_d = nc.dram_tensor("nshift", [B, 1], f32, kind="ExternalInput")
    out_d = nc.dram_tensor("out", [B, CPAD], f32, kind="ExternalOutput")
    st_d = nc.dram_tensor("stats", [128, BT], f32, kind="ExternalOutput")

    with tile.TileContext(nc) as tc:
        with (
            tc.tile_pool(name="xtp", bufs=1) as xtp,
            tc.tile_pool(name="wtp", bufs=4) as wtp,
            tc.tile_pool(name="sbp", bufs=6) as sbp,
            tc.tile_pool(name="expp", bufs=6) as expp,
            tc.tile_pool(name="smp", bufs=1) as smp,
            tc.tile_pool(name="psp", bufs=8, space=bass.MemorySpace.PSUM) as psp,
        ):
            CW = CHUNK_NT * NTILE           # chunk width in classes
            NCH = CPAD // CW                # chunks per core

            def load_chunk(ch):
                wts = [wtp.tile([128, CW], fmm, tag=f"wt{k}", name=f"wt{k}")
                       for k in range(KT)]
                for k in range(KT):
                    nc.sync.dma_start(
                        wts[k][:, :],
                        wt_d[k * 128:(k + 1) * 128, ch * CW:(ch + 1) * CW])
                return wts

            # kick off the first weight chunk before anything else; the small
            # resident loads ride the scalar HWDGE ring concurrently
            wts0 = load_chunk(0)
            xts = [xtp.tile([128, B], fmm, tag=f"xt{k}", name=f"xt{k}") for k in range(KT)]
            for k in range(KT):
                nc.scalar.dma_start(xts[k][:, :], xt_d[k * 128:(k + 1) * 128, :])
            nsh = [smp.tile([128, 1], f32, tag=f"ns{b}", name=f"ns{b}") for b in range(BT)]
            for b in range(BT):
                nc.scalar.dma_start(nsh[b][:, :], ns_d[b * 128:(b + 1) * 128, :])
            zp = [smp.tile([128, CT], f32, tag=f"zp{b}", name=f"zp{b}") for b in range(BT)]

            for ch in range(NCH):
                wts = wts0 if ch == 0 else load_chunk(ch)
                for bt in range(BT):
                    sb = sbp.tile([128, CW], f32, tag="sb", name="sb")
                    for n in range(CHUNK_NT):
                        ct = ch * CHUNK_NT + n
                        ps = psp.tile([128, NTILE], f32, tag="ps", name="ps")
                        for k in range(KT):
                            nc.tensor.matmul(ps[:, :],
                                             xts[k][:, bt * 128:(bt + 1) * 128],
                                             wts[k][:, n * NTILE:(n + 1) * NTILE],
                                             start=(k == 0), stop=(k == KT - 1))
                        nc.vector.tensor_copy(
                            sb[:, n * NTILE:(n + 1) * NTILE], ps[:, :])
                        et = expp.tile([128, NTILE], f32, tag="et", name="et")
                        nc.scalar.activation(
                            et[:, :], ps[:, :], mybir.ActivationFunctionType.Exp,
                            bias=nsh[bt][:, :], scale=1.0,
                            accum_out=zp[bt][:, ct:ct + 1])
                    nc.scalar.dma_start(
                        out_d[bt * 128:(bt + 1) * 128, ch * CW:(ch + 1) * CW],
                        sb[:, :])

            stt = smp.tile([128, BT], f32, tag="stt", name="stt")
            for bt in range(BT):
                nc.vector.tensor_reduce(
                    stt[:, bt:bt + 1], zp[bt][:, :],
                    axis=mybir.AxisListType.X, op=mybir.AluOpType.add)
            nc.sync.dma_start(st_d[:, :], stt[:, :])

    nc.compile()
    return nc


def _get_nc():
    if "nc" not in _CACHE:
        _CACHE["nc"] = _build()
    return _CACHE["nc"]


def kernel(input, label, weight, _want_profile=False):
    from concourse.bass_utils import run_bass_kernel_spmd

    input = np.asarray(input, dtype=np.float32)
    label = np.asarray(label)
    weight = np.asarray(weight, dtype=np.float32)

    # host prep: normalize rows, shard over classes, transpose
    norms = np.maximum(np.sqrt((weight.astype(np.float64) ** 2).sum(1)), 1e-12)
    w_norm = (weight / norms[:, None].astype(np.float32)).astype(np.float32)

    lab = label.astype(np.int64)
    # exact target cosine + ArcFace phi patch value (512 rows, host fp64->fp32)
    cos_t = np.einsum("be,be->b", input.astype(np.float64),
                      w_norm[lab].astype(np.float64))
    sin_t = np.sqrt(np.clip(1.0 - cos_t * cos_t, 0.0, 1.0))
    phi = cos_t * COS_M - sin_t * SIN_M
    patch = np.where(cos_t > TH, phi, cos_t - MM) * S      # [B] fp64
    t_acc = cos_t * S                                      # unpatched target logit

    # numerical-stability shift: row max is ~ S*|x_b|/sqrt(E) * gumbel(C);
    # overshoot a bit so exp never overflows (validity re-checked below).
    rownorm = np.sqrt((input.astype(np.float64) ** 2).sum(1))
    shift = (S * rownorm / math.sqrt(E) * 4.8).astype(np.float32)  # [B]

    if DTYPE_MODE == "bf16":
        import ml_dtypes
        mm_np = ml_dtypes.bfloat16
    elif DTYPE_MODE == "fp16":
        mm_np = np.float16
    else:
        mm_np = np.float32
    xt = np.ascontiguousarray((input * S).T).astype(mm_np)         # [E, B]
    nshift = np.ascontiguousarray((-shift)[:, None]).astype(np.float32)

    in_maps = []
    for c in range(NCORES):
        wsh = w_norm[c * CSH:(c + 1) * CSH]                # [12500, E]
        wt = np.ascontiguousarray(wsh.T).astype(mm_np)
        in_maps.append({"xt": xt, "wt": wt, "nshift": nshift})

    nc = _get_nc()
    try:
        res = run_bass_kernel_spmd(nc, in_maps, core_ids=list(range(NCORES)),
                                   trace=_want_profile)
    except ModuleNotFoundError:
        # no NTFF profiling hook in this container — run untraced
        res = run_bass_kernel_spmd(nc, in_maps, core_ids=list(range(NCORES)))

    out = np.empty((B, C), dtype=np.float32)
    Z = np.zeros(B, dtype=np.float64)
    for c in range(NCORES):
        r = res.results[c]
        out[:, c * CSH:(c + 1) * CSH] = r["out"]
        # stats[p, bt] = Z partial for row bt*128+p
        Z += r["stats"].T.reshape(B).astype(np.float64)

    # swap target element: - exp(t_acc - shift) + exp(patch - shift)
    with np.errstate(over="ignore", invalid="ignore", divide="ignore"):
        Zp = Z - np.exp(t_acc - shift) + np.exp(patch - shift)
        logZ = shift.astype(np.float64) + np.log(Zp)

    bad = ~np.isfinite(logZ)
    if bad.any():
        # fallback: recompute the offending rows from the gathered output
        for b in np.nonzero(bad)[0]:
            row = out[b].astype(np.float64)
            row[lab[b]] = patch[b]
            m = row.max()
            logZ[b] = m + np.log(np.exp(row - m).sum())

    loss = np.float32(-(patch - logZ).mean())

    # scatter the 512 patched target logits
    out[np.arange(B), lab] = patch.astype(np.float32)

    if _want_profile:
        return (out, loss), res
    return (out, loss)
